# revision 5
# baseline (speedup 1.0000x reference)
"""Trainium2 Bass kernel: ActionEmbedder (1x1 conv on spatially-tiled action).

y[b,e] = relu(sum_a action[b,a] * conv_w[e,a] + conv_b[e])
out[b,e,h,w] = y[b,e]  (broadcast over 64x64 spatial positions)

Sharding: data-parallel over batch B=128 across 8 cores (16 rows each);
conv_w/conv_b replicated. Each core computes its 16x256 y block with 4
matmuls, then broadcasts it into [16*256, 4096] rows and streams 64 MiB
to HBM — the kernel is HBM-write-bandwidth bound.
"""

import os
import sys

import numpy as np

B, A, E, H, W = 128, 256, 256, 64, 64
NCORES = 8
BC = B // NCORES  # 16 batch rows per core
HW = H * W  # 4096 spatial positions
ROWS = BC * E  # 4096 output rows per core, each HW f32 long
TILE_F = 2 * HW  # fill-tile free dim: one batch row (= 2 e-halves) per tile


def _ensure_import_path():
    try:
        import concourse.bass  # noqa: F401
    except ImportError:
        for p in ("/opt/trn_rl_repo", os.path.expanduser("~/.axon_site/_ro/trn_rl_repo")):
            if os.path.isdir(p) and p not in sys.path:
                sys.path.insert(0, p)
        import concourse.bass  # noqa: F401


_NC = None


def _build():
    """Build (once) the single-core SPMD Bass program."""
    global _NC
    if _NC is not None:
        return _NC
    _ensure_import_path()
    import concourse.bacc as bacc
    import concourse.mybir as mybir
    import concourse.tile as tile

    fp32 = mybir.dt.float32
    # Bacc (not plain Bass): its compile() runs generate_event_semaphores,
    # which splits multi-wait instructions into EventSemaphore + inst — the
    # TRN2 ISA allows at most one sync wait per regular instruction.
    nc = bacc.Bacc("TRN2", target_bir_lowering=False, debug=False, num_devices=NCORES)

    # All per-core inputs packed into one [128, 546] tensor (single DMA, so
    # downstream matmuls wait on a single DMA semaphore — the PE instruction
    # has very few sync-wait slots). Host-side layout along the free dim:
    #   [0:256)   wT chunk0   wT0[p, e] = conv_w[e, p]        (a = p)
    #   [256:512) wT chunk1   wT1[p, e] = conv_w[e, 128 + p]  (a = 128 + p)
    #   [512:528) actT chunk0 act0[p, b] = action[b, p]
    #   [528:544) actT chunk1 act1[p, b] = action[b, 128 + p]
    #   [544]     bias0[p] = conv_b[p]
    #   [545]     bias1[p] = conv_b[128 + p]
    F_PACKED = 2 * E + 2 * BC + 2
    packed = nc.dram_tensor("packed", [128, F_PACKED], fp32, kind="ExternalInput")
    out = nc.dram_tensor("out", [ROWS, HW], fp32, kind="ExternalOutput")

    with tile.TileContext(nc) as tc:
        with (
            tc.tile_pool(name="const", bufs=1) as cpool,
            tc.tile_pool(name="psum", bufs=1, space="PSUM") as ppool,
            tc.tile_pool(name="fill", bufs=4) as fpool,
        ):
            pk = cpool.tile([128, F_PACKED], fp32, name="pk", tag="pk")
            nc.sync.dma_start(pk[:], packed[:])

            # --- yT[e,b] = relu(w @ action^T + b), e on partitions ---
            # yT columns [h*BC + b] hold y[b, 128h + p] on partition p.
            yT = cpool.tile([128, 2 * BC], fp32, name="yT", tag="yT")
            for h in range(2):  # e-half
                ps = ppool.tile([128, BC], fp32, name=f"ps{h}", tag=f"ps{h}")
                for i in range(2):  # contraction chunk over A
                    nc.tensor.matmul(
                        ps[:],
                        pk[:, i * E + 128 * h : i * E + 128 * (h + 1)],  # lhsT: [K=a, M=e]
                        pk[:, 2 * E + i * BC : 2 * E + (i + 1) * BC],  # rhs: [K=a, N=b]
                        start=(i == 0),
                        stop=(i == 1),
                    )
                nc.scalar.activation(
                    yT[:, h * BC : (h + 1) * BC],
                    ps[:],
                    mybir.ActivationFunctionType.Relu,
                    bias=pk[:, 2 * E + 2 * BC + h : 2 * E + 2 * BC + h + 1],
                    scale=1.0,
                )

            # --- broadcast fill + store: tile t = batch row b=t ---
            # Output row r = b*E + e; e = 128*j + p. Tile free layout [j, f].
            out_ap = out[:]
            for t in range(BC):
                ft = fpool.tile([128, TILE_F], fp32, name=f"ft{t}", tag="fill")
                for h in range(2):
                    col = yT[:, h * BC + t : h * BC + t + 1]  # [128,1] = y[t, 128h+p]
                    dst = ft[:, h * HW : (h + 1) * HW]
                    src = col.broadcast_to([128, HW])
                    if t % 2 == 0:
                        nc.vector.tensor_copy(dst, src)
                    else:
                        nc.scalar.activation(dst, src, mybir.ActivationFunctionType.Copy)
                dst_ap = out_ap[E * t : E * (t + 1), :].rearrange("(j p) f -> p j f", j=2, p=128)
                src_ap = ft[:].rearrange("p (j f) -> p j f", j=2)
                nc.sync.dma_start(dst_ap, src_ap)

    nc.compile()
    _NC = nc
    return nc


def _in_maps(action, conv_w, conv_b):
    action = np.asarray(action, dtype=np.float32)
    wT = np.asarray(conv_w, dtype=np.float32).T  # [A, E]
    bias = np.asarray(conv_b, dtype=np.float32).reshape(E, 1)
    maps = []
    for c in range(NCORES):
        actT = action[c * BC : (c + 1) * BC, :].T  # [A, BC]
        packed = np.concatenate(
            [wT[:128], wT[128:], actT[:128], actT[128:], bias[:128], bias[128:]],
            axis=1,
        )
        maps.append({"packed": np.ascontiguousarray(packed)})
    return maps


def _run_spmd(in_maps, **kwargs):
    _ensure_import_path()
    from concourse.bass_utils import run_bass_kernel_spmd

    nc = _build()
    return run_bass_kernel_spmd(nc, in_maps, list(range(NCORES)), **kwargs)


def kernel(action, conv_w, conv_b):
    br = _run_spmd(_in_maps(action, conv_w, conv_b))
    shards = [br.results[c]["out"].reshape(BC, E, H, W) for c in range(NCORES)]
    return np.concatenate(shards, axis=0)
